# revision 33
# baseline (speedup 1.0000x reference)
"""GATv2 (3-layer, 8-head) message-passing kernel for 8 Trainium2 NeuronCores.

Strategy: partition nodes across the 8 cores by contiguous dst ranges; each
core owns the segment-softmax + aggregation for its nodes.  Edges are grouped
by 128-node dst block and processed in 128-edge tiles:
  - per tile, ONE indirect-DMA gathers x[src] rows (bf16; [128,1] offsets,
    the only shape the HW SWDGE gather ucode supports).  The per-edge DST
    gather is eliminated: dst rows ARE the block's own rows, so a per-block
    xr_blk = xo @ Wr is computed once and the per-edge dst contribution is
    added with an indicator-transpose matmul (itT @ xr_blk) into the same
    PSUM accumulation as the src half (lhsT @ Wl)
  - SBUF->SBUF xbar DMA-transposes (not PE transposes, freeing PSUM banks
    and the PSUM->SBUF copies) build the transposed stationary operands
  - scores (c-major head layout, h innermost, so per-edge-head broadcasts
    hit the 2x DVE mode): one ACT copy of s to bf16, DVE relu, att-mult,
    channel reduce; the 0.2*s linear part of leaky-relu comes from tiny PE
    matmuls (att-folded weights) and is folded in as
    exp(e) = exp(e_relu) * exp(e_lin) so the lin PSUM bank frees early
  - aggregation is matmul-based scatter: an indicator matrix (dst-one-hot,
    one is_equal against an iota constant, on GPSIMD) contracts ex-weighted
    s into per-block PSUM accumulators; softmax normalisation and the GATv2
    "alpha*xl = alpha*s - xr" identity are applied per block, followed by
    head-mean, residual, LayerNorm (rsqrt via Ln+Exp so the whole kernel
    stays in one ACT table set) and relu
  - emission is software-pipelined in three stages per group with all
    no-late-dependency ops (gathers, indicators, transposes) hoisted one
    block ahead, so in-order engine queues never wait on same-group results
Between layers the updated bf16 node features are exchanged with a device
AllGather.  All per-core variation is carried in input data (edge indices),
so one SPMD program runs on all 8 cores.
"""

import sys, os

for _p in ("/opt/trn_rl_repo", "/root/.axon_site/_ro/trn_rl_repo"):
    if os.path.isdir(_p) and _p not in sys.path:
        sys.path.append(_p)

import numpy as np
import ml_dtypes

import concourse.bass as bass
import concourse.mybir as mybir
import concourse.tile as tile
from concourse.bass_utils import run_bass_kernel_spmd
from concourse.vector_clock import ScopedClock

BF16 = mybir.dt.bfloat16
F32 = mybir.dt.float32
I32 = mybir.dt.int32
AF = mybir.ActivationFunctionType
ALU = mybir.AluOpType

NEG_SLOPE = 0.2
LN_EPS = 1e-5
GSZ_V = 1
AG_CHUNKS = 1       # AllGather split count (overlap tail of layer with AG)
INDT_POOL = True    # build dst one-hot on GPSIMD instead of DVE
TRANS_DMA = True    # xbar DMA transpose instead of PE transpose + copy
MW2_POOL = False    # att-mult on GPSIMD instead of DVE

# ---------------------------------------------------------------------------
# TileContext tail-drain patch: this container's walrus rejects a Drain that
# carries many semaphore waits ("Too many sync wait commands").  Split the
# kernel-tail drain's waits into individual wait_ge instructions.
_MAXW = 1


def _patched_drain_and_barrier(self, tick_clock, wait_clock):
    nc = self.nc
    probe = nc.sync.drain()
    wait_clock.add_sem_waits(probe.ins, ScopedClock({None: tick_clock.global_clock}))
    waits = list(probe.ins.sync_info.on_wait)
    if len(waits) > _MAXW:
        bb = nc.cur_bb.bb
        insts = bb.instructions
        assert insts[-1].name == probe.ins.name
        insts.pop()
        by_num = {h.num: h for h in self.sems.allocated().values()}
        for w in waits:
            assert w.wait_mode == "sem-ge-imm", w
            h = by_num.get(w.id)
            assert h is not None, (w.id, w.ant_name)
            nc.sync.wait_ge(h, w.wait_value)
        nc.sync.drain()
    nc.all_engine_barrier()
    assert self.sems is not None
    popped = nc._tile_sem_poison_stack.pop()
    assert popped is self._sem_poison
    nc.clear_and_free_semaphores(list(self.sems.allocated().values()))
    nc.all_engine_barrier()


tile.TileContext._drain_and_barrier = _patched_drain_and_barrier

import bass_rust as _bass_rust


def split_excess_waits(nc):
    """This container's walrus accepts only 1 sem wait on most instruction
    structs (2 on EventSemaphore, ~3 on CTRL).  Hoist excess waits into
    standalone EventSemaphore instructions inserted just before the
    overloaded instruction on the same engine."""
    wid = [0]
    for f in nc.m.functions:
        for bbl in f.blocks:
            insts = bbl.instructions
            i = 0
            while i < len(insts):
                inst = insts[i]
                si = inst.sync_info
                if si is None:
                    i += 1
                    continue
                tname = type(inst).__name__
                cap = 2 if tname == "InstEventSemaphore" else 1
                waits = list(si.on_wait)
                if len(waits) <= cap:
                    i += 1
                    continue
                keep = waits[-cap:]
                hoist = waits[:-cap]
                inst.sync_info = _bass_rust.SyncInfo(
                    on_wait=keep, on_update=list(si.on_update)
                )
                pos = i
                for j in range(0, len(hoist), 2):
                    ev = mybir.InstEventSemaphore(
                        name=f"WX-{wid[0]}", ins=[], outs=[]
                    )
                    wid[0] += 1
                    ev.engine = inst.engine
                    ev.sync_info = _bass_rust.SyncInfo(
                        on_wait=hoist[j : j + 2], on_update=[]
                    )
                    insts.insert(pos, ev)
                    pos += 1
                    i += 1
                i += 1


class Cfg:
    def __init__(self, n_nodes=50000, n_edges=400000, ncore=8, d=64, h=8, nlayer=3):
        self.N = n_nodes
        self.E = n_edges
        self.NCORE = ncore
        self.D = d
        self.H = h
        self.L = nlayer
        self.PER = n_nodes // ncore          # owned real nodes per core
        self.BLK = 128
        self.NB = -(-self.PER // self.BLK)    # blocks per core
        self.PPAD = self.NB * self.BLK        # padded nodes per core
        self.HC = h * d                       # 512


# ---------------------------------------------------------------------------
# Host-side graph partitioning


def partition_edges(cfg, edge_index):
    """Returns (eidx [NCORE, NT, 3, 128] int32, TB) with per-(core,block)
    128-edge tiles.  Channel 0: padded-global src id, channel 1: padded-global
    dst id, channel 2: dst column within block as f32 bits (-1 for pad)."""
    N, PER, PPAD, BLK, NB = cfg.N, cfg.PER, cfg.PPAD, cfg.BLK, cfg.NB
    src = np.concatenate([edge_index[0], np.arange(N, dtype=np.int64)]).astype(np.int64)
    dst = np.concatenate([edge_index[1], np.arange(N, dtype=np.int64)]).astype(np.int64)
    srcp = (src // PER) * PPAD + src % PER
    owner = dst // PER
    loc = dst % PER
    blk = loc // BLK
    dcol = loc % BLK

    key = (owner * NB + blk).astype(np.int64)
    order = np.argsort(key, kind="stable")
    key_s = key[order]
    counts = np.bincount(key_s, minlength=cfg.NCORE * NB)
    # pad nodes (loc >= PER within the last block) get one fake self edge so
    # their softmax denominator is finite
    npad_node = PPAD - PER
    maxcnt = int(counts.max()) + (npad_node if npad_node else 0)
    TB = max(1, -(-maxcnt // 128))
    if TB % GSZ_V:
        TB += GSZ_V - TB % GSZ_V  # keep groups block-aligned
    NT = NB * TB

    eidx = np.zeros((cfg.NCORE, NT, 3, 128), np.int32)
    eidx[:, :, 2, :] = np.float32(-1.0).view(np.int32)  # pad dcol = -1.0f
    starts = np.zeros(cfg.NCORE * NB + 1, np.int64)
    np.cumsum(counts, out=starts[1:])
    srcp_s = srcp[order]
    dstp_s = (owner * PPAD + loc)[order]
    dcol_s = dcol[order]
    for k in range(cfg.NCORE):
        for b in range(NB):
            s0, s1 = starts[k * NB + b], starts[k * NB + b + 1]
            cnt = int(s1 - s0)
            es = srcp_s[s0:s1]
            ed = dstp_s[s0:s1]
            ec = dcol_s[s0:s1].astype(np.float32)
            if b == NB - 1 and npad_node:
                # fake self edges for pad nodes
                pad_ids = k * PPAD + np.arange(PER, PPAD, dtype=np.int64)
                es = np.concatenate([es, pad_ids])
                ed = np.concatenate([ed, pad_ids])
                ec = np.concatenate(
                    [ec, np.arange(PER - (NB - 1) * BLK, BLK, dtype=np.float32)]
                )
                cnt += npad_node
            nslot = TB * 128
            fsrc = np.zeros(nslot, np.int32)
            fdst = np.zeros(nslot, np.int32)
            fcol = np.full(nslot, np.float32(-1.0).view(np.int32), np.int32)
            fsrc[:cnt] = es
            fdst[:cnt] = ed
            fcol[:cnt] = ec.view(np.int32)
            eidx[k, b * TB : (b + 1) * TB, 0, :] = fsrc.reshape(TB, 128)
            eidx[k, b * TB : (b + 1) * TB, 1, :] = fdst.reshape(TB, 128)
            eidx[k, b * TB : (b + 1) * TB, 2, :] = fcol.reshape(TB, 128)
    return eidx, TB


# ---------------------------------------------------------------------------
# Device program


def build_program(cfg, TB, trivial_affine=True, with_cc=True, ablate=(), reps=1):
    NB, BLK, D, H, HC, L = cfg.NB, cfg.BLK, cfg.D, cfg.H, cfg.HC, cfg.L
    NT = NB * TB
    PPAD = cfg.PPAD
    NPADG = cfg.NCORE * PPAD
    PER = cfg.PER
    NCORE = cfg.NCORE
    n = GSZ_V
    assert TB % n == 0

    nc = bass.Bass("TRN2", target_bir_lowering=False, debug=False, num_devices=NCORE)

    xin = nc.dram_tensor("xin", [NPADG, D], BF16, kind="ExternalInput").ap()
    xown0 = nc.dram_tensor("xown0", [PPAD, D], BF16, kind="ExternalInput").ap()
    eidx = nc.dram_tensor("eidx", [NT, 3, 128], I32, kind="ExternalInput").ap()
    wc = nc.dram_tensor("wc", [L, 2 * D, HC], BF16, kind="ExternalInput").ap()
    att2 = nc.dram_tensor("att2", [L, 128, n * HC], BF16, kind="ExternalInput").ap()
    wrmn = nc.dram_tensor("wrmn", [L, D, D], BF16, kind="ExternalInput").ap()
    wcl = nc.dram_tensor("wcl", [L, 2 * D, H], BF16, kind="ExternalInput").ap()
    id64 = nc.dram_tensor("id64", [D, D], BF16, kind="ExternalInput").ap()
    id128 = nc.dram_tensor("id128", [128, 128], BF16, kind="ExternalInput").ap()
    iotar = nc.dram_tensor("iotar", [128, 128], F32, kind="ExternalInput").ap()
    out = nc.dram_tensor("out", [PER, D], F32, kind="ExternalOutput").ap()

    with tile.TileContext(nc) as tc:
        import contextlib

        ctx = contextlib.ExitStack()
        with ctx:
            cpool = ctx.enter_context(tc.tile_pool(name="consts", bufs=1))
            dpool = ctx.enter_context(tc.tile_pool(name="dram", bufs=1, space="DRAM"))
            ppool = ctx.enter_context(tc.tile_pool(name="psum", bufs=1, space="PSUM"))
            spool = ctx.enter_context(tc.tile_pool(name="sbuf", bufs=2 * 10 + 2))
            bpool = ctx.enter_context(tc.tile_pool(name="blk", bufs=2))
            gpool = ctx.enter_context(tc.tile_pool(name="grp", bufs=3))
            fpool = ctx.enter_context(tc.tile_pool(name="fin", bufs=3))

            # persistent constants
            iota_sb = cpool.tile([128, 128], F32, tag="iotar")
            nc.sync.dma_start(out=iota_sb[:], in_=iotar[:])
            id64_sb = cpool.tile([D, D], BF16, tag="id64")
            nc.sync.dma_start(out=id64_sb[:], in_=id64[:])
            id128_sb = cpool.tile([128, 128], BF16, tag="id128")
            if not TRANS_DMA:
                nc.sync.dma_start(out=id128_sb[:], in_=id128[:])
            epsc = cpool.tile([128, 1], F32, tag="epsc")
            nc.gpsimd.memset(epsc[:], LN_EPS)
            wl_sb, wr_sb, att2_sb, wrmn_sb, wcll_sb, wclr_sb = [], [], [], [], [], []
            for l in range(L):
                w0 = cpool.tile([D, HC], BF16, tag=f"wl{l}")
                nc.sync.dma_start(out=w0[:], in_=wc[l, :D])
                wl_sb.append(w0)
                w1 = cpool.tile([D, HC], BF16, tag=f"wr{l}")
                nc.sync.dma_start(out=w1[:], in_=wc[l, D:])
                wr_sb.append(w1)
                a = cpool.tile([128, n * HC], BF16, tag=f"att2{l}")
                nc.sync.dma_start(out=a[:], in_=att2[l])
                att2_sb.append(a)
                m = cpool.tile([D, D], BF16, tag=f"wrmn{l}")
                nc.sync.dma_start(out=m[:], in_=wrmn[l])
                wrmn_sb.append(m)
                c0 = cpool.tile([D, H], BF16, tag=f"wcll{l}", name=f"wcll{l}")
                nc.sync.dma_start(out=c0[:], in_=wcl[l, :D])
                wcll_sb.append(c0)
                c1 = cpool.tile([D, H], BF16, tag=f"wclr{l}", name=f"wclr{l}")
                nc.sync.dma_start(out=c1[:], in_=wcl[l, D:])
                wclr_sb.append(c1)

            # inter-layer DRAM buffers
            xsh = [
                dpool.tile([PPAD, D], BF16, tag=f"xsh{i}", name=f"xsh{i}")
                for i in range(L - 1)
            ]
            xg = [
                dpool.tile(
                    [NPADG, D], BF16, tag=f"xg{i}", name=f"xg{i}",
                    addr_space="Shared" if reps == 1 else "Local",
                )
                for i in range(L - 1)
            ]

            NLIN = H * n  # lin region width in the merged dxl bank

            def finalize(l, b, ctx_b):
                agg_ps = ctx_b["agg"]
                dxa = ctx_b["dxa"]
                den_ps = dxa[:, :H]
                xoT = ctx_b["xoT"]
                recip = fpool.tile([128, H], F32, tag="recip")
                nc.vector.reciprocal(out=recip[:], in_=den_ps)
                # agg layout is c-major [p, (c h)]
                tmp = fpool.tile([128, HC], BF16, tag="ftmp")
                rb = recip[:].unsqueeze(1).to_broadcast([128, D, H])
                nc.vector.tensor_tensor(
                    out=tmp[:], in0=agg_ps[:].rearrange("p (c h) -> p c h", h=H),
                    in1=rb, op=ALU.mult
                )
                v = fpool.tile([128, D], F32, tag="v")
                tmp_t = tmp[:].rearrange("p (c h) -> p c h", h=H)
                nc.vector.tensor_reduce(
                    out=v[:], in_=tmp_t, axis=mybir.AxisListType.X, op=ALU.add
                )
                xadj = dxa[:, 16 : 16 + D]
                nc.tensor.matmul(
                    out=xadj, lhsT=xoT[:], rhs=wrmn_sb[l][:],
                    start=True, stop=False, skip_group_check=True,
                )
                nc.tensor.matmul(
                    out=xadj, lhsT=xoT[:], rhs=id64_sb[:],
                    start=False, stop=True, skip_group_check=True,
                )
                # t3 = v/H + xadj  (head mean + residual - Wr head-mean corr)
                t3 = fpool.tile([128, D], F32, tag="t3")
                nc.vector.scalar_tensor_tensor(
                    out=t3[:], in0=v[:], scalar=1.0 / H, in1=xadj,
                    op0=ALU.mult, op1=ALU.add,
                )
                scr = fpool.tile([128, D], F32, tag="scr")
                mu = fpool.tile([128, 1], F32, tag="mu")
                nc.scalar.activation(
                    out=scr[:], in_=t3[:], func=AF.Copy, scale=1.0 / D, accum_out=mu[:]
                )
                cen = fpool.tile([128, D], F32, tag="cen")
                nc.vector.tensor_scalar(
                    out=cen[:], in0=t3[:], scalar1=mu[:, 0:1], scalar2=None,
                    op0=ALU.subtract,
                )
                scr2 = fpool.tile([128, D], F32, tag="scr2")
                vs = fpool.tile([128, 1], F32, tag="vs")
                nc.scalar.activation(
                    out=scr2[:], in_=cen[:], func=AF.Square, accum_out=vs[:]
                )
                # rsqrt(var+eps) = exp(-0.5*ln(var+eps)); Ln/Exp/Copy/Square/
                # Relu share one ACT table set (no table reloads)
                lnv = fpool.tile([128, 1], F32, tag="lnv")
                nc.scalar.activation(
                    out=lnv[:], in_=vs[:], func=AF.Ln, scale=1.0 / D,
                    bias=epsc[:, 0:1],
                )
                rs = fpool.tile([128, 1], F32, tag="rs")
                nc.scalar.activation(out=rs[:], in_=lnv[:], func=AF.Exp, scale=-0.5)
                xn = fpool.tile([128, D], F32, tag="xn")
                nc.vector.tensor_scalar(
                    out=xn[:], in0=cen[:], scalar1=rs[:, 0:1], scalar2=None,
                    op0=ALU.mult,
                )
                if l < L - 1:
                    y = fpool.tile([128, D], BF16, tag="yb")
                    nc.scalar.activation(out=y[:], in_=xn[:], func=AF.Relu)
                    nc.sync.dma_start(
                        out=xsh[l][b * BLK : (b + 1) * BLK], in_=y[:]
                    )
                else:
                    y = fpool.tile([128, D], F32, tag="yf")
                    nc.scalar.activation(out=y[:], in_=xn[:], func=AF.Relu)
                    rows = min(BLK, PER - b * BLK)
                    if rows > 0:
                        nc.sync.dma_start(
                            out=out[b * BLK : b * BLK + rows], in_=y[:rows, :]
                        )

            for _rep in range(reps):
              for l in range(L):
                xsrc = xin if l == 0 else xg[l - 1][:]
                xown_src = xown0 if l == 0 else xsh[l - 1][:]
                ag_done = 0

                def prolog(b, l=l, xsrc=xsrc, xown_src=xown_src):
                    """Emit the no-late-dependency ops of block b: edge table,
                    src gathers, indicators + their transposes, own-node
                    transform.  Emitted one block AHEAD of processing so the
                    Pool/SP queues never sit behind late chain ops (mw2)."""
                    ei = bpool.tile([128, 3 * TB], I32, tag="ei")
                    nc.sync.dma_start(
                        out=ei[:],
                        in_=eidx[b * TB : (b + 1) * TB].rearrange("t c p -> p (t c)"),
                    )
                    ei3 = ei[:].rearrange("p (t c) -> p t c", c=3)
                    xo = fpool.tile([128, 2 * D], BF16, tag="xo")
                    nc.sync.dma_start(
                        out=xo[:, :D], in_=xown_src[b * BLK : (b + 1) * BLK]
                    )
                    nc.gpsimd.memset(xo[:, D:], 0)
                    xoT128 = fpool.tile([128, 128], BF16, tag="xoT")
                    nc.sync.dma_start(out=xoT128[:], in_=xo[:], transpose=True)
                    xsd_t, ind_t, itT_t, lhsT_t = [], [], [], []
                    for t in range(TB):
                        xsd = spool.tile([128, 128], BF16, tag="xsd")
                        nc.gpsimd.indirect_dma_start(
                            out=xsd[:, 0:D], out_offset=None, in_=xsrc,
                            in_offset=bass.IndirectOffsetOnAxis(
                                ap=ei3[:, t, 0:1], axis=0
                            ),
                        )
                        nc.gpsimd.memset(xsd[:, D:], 0)
                        it = spool.tile([128, 128], BF16, tag="indT")
                        eng = nc.gpsimd if INDT_POOL else nc.vector
                        eng.tensor_scalar(
                            out=it[:], in0=iota_sb[:],
                            scalar1=ei3[:, t, 2:3].bitcast(F32),
                            scalar2=None, op0=ALU.is_equal,
                        )
                        itT = spool.tile([128, 128], BF16, tag="itT")
                        nc.sync.dma_start(out=itT[:], in_=it[:], transpose=True)
                        lhsT = spool.tile([128, 128], BF16, tag="lhsT")
                        nc.sync.dma_start(out=lhsT[:], in_=xsd[:], transpose=True)
                        xsd_t.append(xsd)
                        ind_t.append(it)
                        itT_t.append(itT)
                        lhsT_t.append(lhsT)
                    # xr_blk = xo @ Wr (+ its lin part) replaces the per-edge
                    # dst gather; briefly borrows a slot of the s2 PSUM ring
                    xoT = xoT128[:D, :]
                    xrbp = ppool.tile([128, HC * n], F32, tag="s2", bufs=2)
                    nc.tensor.matmul(
                        out=xrbp[:, :HC], lhsT=xoT, rhs=wr_sb[l][:],
                        start=True, stop=True, skip_group_check=True,
                    )
                    lrbp = ppool.tile([128, NLIN], F32, tag="lin", bufs=2)
                    nc.tensor.matmul(
                        out=lrbp[:, :H], lhsT=xoT, rhs=wclr_sb[l][:],
                        start=True, stop=True, skip_group_check=True,
                    )
                    xrb = fpool.tile([128, HC], BF16, tag="xrb")
                    nc.scalar.copy(out=xrb[:], in_=xrbp[:, :HC])
                    lrb = fpool.tile([128, H], BF16, tag="lrb")
                    nc.scalar.copy(out=lrb[:], in_=lrbp[:, :H])
                    return dict(ind_t=ind_t, itT_t=itT_t, lhsT_t=lhsT_t,
                                xoT=xoT, xrb=xrb, lrb=lrb)

                # ---- fully software-pipelined group stream: stage A of
                # group q runs alongside stage B1 of q-1 and B2 of q-2, so
                # no in-order engine queue ever waits on a same-group result
                NGB = TB // n
                NQ = NB * NGB
                pros = {0: prolog(0)}
                blkst = {}
                qst = {}

                def stageA(q):
                    b, k = divmod(q, NGB)
                    if k == 0:
                        if b + 1 < NB:
                            pros[b + 1] = prolog(b + 1)
                        agg_t = ppool.tile([128, HC], F32, tag="agg", bufs=1,
                                           name=f"agg{l}_{b}")
                        dxa_t = ppool.tile([128, 16 + D], F32, tag="dxa",
                                           bufs=1, name=f"dxa{l}_{b}")
                        blkst[b] = dict(agg=agg_t, dxa=dxa_t, pro=pros.pop(b))
                    st = blkst[b]
                    pro = st["pro"]
                    st["xoT"] = pro["xoT"]
                    g = k * n
                    s2 = ppool.tile([128, HC * n], F32, tag="s2", bufs=2)
                    lint = ppool.tile([128, NLIN], F32, tag="lin", bufs=2)
                    lin = lint[:]
                    for j in range(n):
                        t = g + j
                        itT = pro["itT_t"][t]
                        lhsT = pro["lhsT_t"][t]
                        # s = xl[src] + xr[dst]: src half via Wl, dst half
                        # via indicator-gather of xr_blk rows
                        nc.tensor.matmul(
                            out=s2[:, HC * j : HC * (j + 1)],
                            lhsT=lhsT[:D, :], rhs=wl_sb[l][:],
                            start=True, stop=False, skip_group_check=True,
                        )
                        nc.tensor.matmul(
                            out=s2[:, HC * j : HC * (j + 1)],
                            lhsT=itT[:], rhs=pro["xrb"][:],
                            start=False, stop=True, skip_group_check=True,
                        )
                        nc.tensor.matmul(
                            out=lin[:, H * j : H * (j + 1)],
                            lhsT=lhsT[:D, :], rhs=wcll_sb[l][:],
                            start=True, stop=False, skip_group_check=True,
                        )
                        nc.tensor.matmul(
                            out=lin[:, H * j : H * (j + 1)],
                            lhsT=itT[:], rhs=pro["lrb"][:],
                            start=False, stop=True, skip_group_check=True,
                        )
                    # one bf16 copy of s feeds both score + agg paths
                    s2sb = gpool.tile([128, HC * n], BF16, tag="s2sb")
                    nc.scalar.copy(out=s2sb[:], in_=s2[:, : HC * n])
                    m2 = gpool.tile([128, HC * n], BF16, tag="m2")
                    nc.vector.tensor_scalar(
                        out=m2[:], in0=s2sb[:], scalar1=0.0, scalar2=None,
                        op0=ALU.max,
                    )
                    qst[q] = dict(s2sb=s2sb, m2=m2, lin=lin)

                def stageB1(q):
                    s = qst[q]
                    exl = gpool.tile([128, H * n], BF16, tag="exl")
                    nc.scalar.activation(out=exl[:], in_=s["lin"], func=AF.Exp)
                    mw2 = gpool.tile([128, HC * n], BF16, tag="mw2")
                    mweng = nc.gpsimd if MW2_POOL else nc.vector
                    mweng.tensor_tensor(
                        out=mw2[:], in0=s["m2"][:], in1=att2_sb[l][:],
                        op=ALU.mult,
                    )
                    e2r = gpool.tile([128, H * n], BF16, tag="e2r")
                    with nc.allow_low_precision(reason="score sums in bf16"):
                        nc.vector.tensor_reduce(
                            out=e2r[:],
                            in_=mw2[:].rearrange("p (t c h) -> p t h c",
                                                 t=n, h=H),
                            axis=mybir.AxisListType.X, op=ALU.add,
                        )
                    s["exl"] = exl
                    s["e2r"] = e2r

                def stageB2(q):
                    b, k = divmod(q, NGB)
                    s = qst.pop(q)
                    st = blkst[b]
                    g = k * n
                    exr = gpool.tile([128, H * n], BF16, tag="exr")
                    nc.scalar.activation(out=exr[:], in_=s["e2r"][:],
                                         func=AF.Exp)
                    # exp(e2r + lin) = exp(e2r) * exp(lin)
                    ex2 = gpool.tile([128, H * n], BF16, tag="ex2")
                    nc.vector.tensor_tensor(
                        out=ex2[:], in0=exr[:], in1=s["exl"][:], op=ALU.mult
                    )
                    w2 = gpool.tile([128, HC * n], BF16, tag="w2")
                    exb = (
                        ex2[:]
                        .rearrange("p (t h) -> p t h", t=n)
                        .unsqueeze(2)
                        .to_broadcast([128, n, D, H])
                    )
                    nc.vector.tensor_tensor(
                        out=w2[:], in0=s["s2sb"][:], in1=exb, op=ALU.mult
                    )
                    for j in range(n):
                        tt = g + j
                        indTj = st["pro"]["ind_t"][tt]
                        nc.tensor.matmul(
                            out=st["agg"][:], lhsT=indTj[:],
                            rhs=w2[:, HC * j : HC * (j + 1)],
                            start=(tt == 0), stop=(tt == TB - 1),
                            skip_group_check=True,
                        )
                        nc.tensor.matmul(
                            out=st["dxa"][:, :H], lhsT=indTj[:],
                            rhs=ex2[:, H * j : H * (j + 1)],
                            start=(tt == 0), stop=(tt == TB - 1),
                            skip_group_check=True,
                        )
                    if k == NGB - 1:
                        finalize(l, b, blkst.pop(b))
                        # chunked AllGather: ship finished rows early
                        nonlocal_state = None
                        if l < L - 1 and with_cc and AG_CHUNKS > 1:
                            nonlocal ag_done
                            cb = (ag_done + 1) * NB // AG_CHUNKS
                            if b + 1 >= cb and ag_done < AG_CHUNKS - 1:
                                r0 = ag_done * NB // AG_CHUNKS * BLK
                                r1 = cb * BLK
                                xgv = xg[l][:].rearrange(
                                    "(k p) d -> k p d", k=NCORE
                                )[:, r0:r1, :]
                                nc.gpsimd.collective_compute(
                                    "AllGather", ALU.bypass,
                                    replica_groups=[list(range(NCORE))],
                                    ins=[xsh[l][r0:r1]], outs=[xgv],
                                )
                                ag_done += 1

                for q in range(NQ + 2):
                    # virtual-time buckets force the Tile scheduler to keep
                    # the software-pipeline interleaving per engine
                    if q < NQ:
                        stageA(q)
                    if 1 <= q <= NQ:
                        stageB1(q - 1)
                    if 2 <= q <= NQ + 1:
                        stageB2(q - 2)
                if l < L - 1 and with_cc:
                    if AG_CHUNKS > 1:
                        r0 = ag_done * NB // AG_CHUNKS * BLK
                        xgv = xg[l][:].rearrange(
                            "(k p) d -> k p d", k=NCORE
                        )[:, r0:PPAD, :]
                        nc.gpsimd.collective_compute(
                            "AllGather", ALU.bypass,
                            replica_groups=[list(range(NCORE))],
                            ins=[xsh[l][r0:PPAD]], outs=[xgv],
                        )
                    else:
                        nc.gpsimd.collective_compute(
                            "AllGather", ALU.bypass,
                            replica_groups=[list(range(NCORE))],
                            ins=[xsh[l].opt()], outs=[xg[l].opt()],
                        )
    return nc


# ---------------------------------------------------------------------------
# Host wrapper


def prep_inputs(cfg, x, Wl, Wr, att, edge_index):
    bf = ml_dtypes.bfloat16
    D, H, HC, L = cfg.D, cfg.H, cfg.HC, cfg.L
    PER, PPAD = cfg.PER, cfg.PPAD
    eidx, TB = partition_edges(cfg, edge_index.astype(np.int64))

    xpad = np.zeros((cfg.NCORE * PPAD, D), np.float32)
    for k in range(cfg.NCORE):
        xpad[k * PPAD : k * PPAD + PER] = x[k * PER : (k + 1) * PER]
    xpad_bf = xpad.astype(bf)

    # c-major head layout: output column (c*H + h) holds head h, channel c
    wc = np.concatenate([Wl, Wr], axis=1).reshape(L, 2 * D, H, D)
    wc = np.ascontiguousarray(wc.transpose(0, 1, 3, 2)).reshape(L, 2 * D, HC)
    wc = wc.astype(bf)
    attf = (0.8 * att.transpose(0, 2, 1)).reshape(L, 1, HC)  # [c, h] flat
    att2 = np.broadcast_to(
        np.concatenate([attf] * GSZ_V, axis=2), (L, 128, GSZ_V * HC)
    ).astype(bf)
    # lin[d, h] = 0.2 * sum_c Wc[d, h*CH+c] * att[h, c]
    wcf = np.concatenate([Wl, Wr], axis=1).reshape(L, 2 * D, H, D)
    wcl = (0.2 * np.einsum('ldhc,lhc->ldh', wcf, att)).astype(bf)  # [L, 128, 8]
    wrmn = (-Wr.reshape(L, D, H, D).mean(axis=2)).astype(bf)  # negated head-mean
    id64 = np.eye(D, dtype=np.float32).astype(bf)
    id128 = np.eye(128, dtype=np.float32).astype(bf)
    iotar = np.broadcast_to(
        np.arange(128, dtype=np.float32)[None, :], (128, 128)
    ).copy()

    shared = dict(
        wc=np.ascontiguousarray(wc), att2=np.ascontiguousarray(att2),
        wcl=np.ascontiguousarray(wcl),
        wrmn=np.ascontiguousarray(wrmn), id64=id64, id128=id128, iotar=iotar,
        xin=xpad_bf,
    )
    in_maps = []
    for k in range(cfg.NCORE):
        m = dict(shared)
        m["eidx"] = np.ascontiguousarray(eidx[k])
        m["xown0"] = np.ascontiguousarray(xpad_bf[k * PPAD : (k + 1) * PPAD])
        in_maps.append(m)
    return in_maps, TB


_CACHE = {}


def _get_program(cfg_key, cfg, TB):
    key = (cfg_key, TB)
    if key not in _CACHE:
        nc = build_program(cfg, TB)
        split_excess_waits(nc)  # walrus-only fixup; breaks CoreSim bookkeeping
        _CACHE[key] = nc
    return _CACHE[key]


def kernel(x, Wl, Wr, att, b, gamma, beta, edge_index, _return_extras=False):
    x = np.asarray(x, np.float32)
    Wl = np.asarray(Wl, np.float32)
    Wr = np.asarray(Wr, np.float32)
    att = np.asarray(att, np.float32)
    edge_index = np.asarray(edge_index)
    assert np.all(np.asarray(b) == 0.0), "nonzero bias not supported"
    assert np.all(np.asarray(gamma) == 1.0), "non-unit gamma not supported"
    assert np.all(np.asarray(beta) == 0.0), "nonzero beta not supported"

    cfg = Cfg(
        n_nodes=x.shape[0], n_edges=edge_index.shape[1], d=x.shape[1],
        h=att.shape[1], nlayer=Wl.shape[0],
    )
    in_maps, TB = prep_inputs(cfg, x, Wl, Wr, att, edge_index)
    nc = _get_program((cfg.N, cfg.E, cfg.D, cfg.H, cfg.L), cfg, TB)
    res = run_bass_kernel_spmd(nc, in_maps, core_ids=list(range(cfg.NCORE)))
    outp = np.concatenate([res.results[k]["out"] for k in range(cfg.NCORE)], axis=0)
    outp = outp.astype(np.float32)
    if _return_extras:
        return outp, res
    return outp


# revision 36
# speedup vs baseline: 2.4083x; 2.4083x over previous
"""GATv2 (3-layer, 8-head) message-passing kernel for 8 Trainium2 NeuronCores.

Strategy: partition nodes across the 8 cores by contiguous dst ranges; each
core owns the segment-softmax + aggregation for its nodes.  Edges are grouped
by 128-node dst block and processed in 128-edge tiles:
  - indirect-DMA gather of x[src], x[dst] rows (bf16; [128,1] offsets, the
    only shape the HW SWDGE gather ucode supports)
  - one PE transpose builds the combined [x_src^T ; x_dst^T] stationary
    operand; one matmul against [Wl ; Wr] yields s = xl[src] + xr[dst],
    in a c-major head layout (column c*H + h) so per-edge-head broadcasts
    keep h innermost and hit the 2x DVE mode
  - scores: one ACT copy of s to bf16, then DVE-side relu (tensor_scalar,
    2x) * att + channel reduce + exp; the 0.2*s linear part of leaky-relu
    is a tiny PE matmul (att-folded weights) whose PSUM the DVE adds
    directly (no ACT staging copy)
  - aggregation is matmul-based scatter: an indicator matrix (dst-one-hot,
    built with one tensor_scalar is_equal against an iota constant) contracts
    ex-weighted s into per-block PSUM accumulators; softmax normalisation and
    the GATv2 "alpha*xl = alpha*s - xr" identity are applied per block,
    followed by head-mean, residual, LayerNorm and relu.  rsqrt is computed
    as exp(-0.5*ln(var+eps)) so the whole kernel stays in one ACT table set
    (exp and sqrt never share one; reloads cost 1.3us per switch).
Between layers the updated bf16 node features are exchanged with a device
AllGather.  All per-core variation is carried in input data (edge indices),
so one SPMD program runs on all 8 cores.
"""

import sys, os

for _p in ("/opt/trn_rl_repo", "/root/.axon_site/_ro/trn_rl_repo"):
    if os.path.isdir(_p) and _p not in sys.path:
        sys.path.append(_p)

import numpy as np
import ml_dtypes

import concourse.bass as bass
import concourse.mybir as mybir
import concourse.tile as tile
from concourse.bass_utils import run_bass_kernel_spmd
from concourse.vector_clock import ScopedClock

BF16 = mybir.dt.bfloat16
F32 = mybir.dt.float32
I32 = mybir.dt.int32
AF = mybir.ActivationFunctionType
ALU = mybir.AluOpType

NEG_SLOPE = 0.2
LN_EPS = 1e-5
# PSUM slot counts (8 banks total), tunable for model probing
B_TPS, B_AGG, B_DX, B_S2, B_LIN = 2, 2, 1, 1, 1
GSZ_V = 2

# ---------------------------------------------------------------------------
# TileContext tail-drain patch: this container's walrus rejects a Drain that
# carries many semaphore waits ("Too many sync wait commands").  Split the
# kernel-tail drain's waits into individual wait_ge instructions.
_MAXW = 1


def _patched_drain_and_barrier(self, tick_clock, wait_clock):
    nc = self.nc
    probe = nc.sync.drain()
    wait_clock.add_sem_waits(probe.ins, ScopedClock({None: tick_clock.global_clock}))
    waits = list(probe.ins.sync_info.on_wait)
    if len(waits) > _MAXW:
        bb = nc.cur_bb.bb
        insts = bb.instructions
        assert insts[-1].name == probe.ins.name
        insts.pop()
        by_num = {h.num: h for h in self.sems.allocated().values()}
        for w in waits:
            assert w.wait_mode == "sem-ge-imm", w
            h = by_num.get(w.id)
            assert h is not None, (w.id, w.ant_name)
            nc.sync.wait_ge(h, w.wait_value)
        nc.sync.drain()
    nc.all_engine_barrier()
    assert self.sems is not None
    popped = nc._tile_sem_poison_stack.pop()
    assert popped is self._sem_poison
    nc.clear_and_free_semaphores(list(self.sems.allocated().values()))
    nc.all_engine_barrier()


tile.TileContext._drain_and_barrier = _patched_drain_and_barrier

import bass_rust as _bass_rust


def split_excess_waits(nc):
    """This container's walrus accepts only 1 sem wait on most instruction
    structs (2 on EventSemaphore, ~3 on CTRL).  Hoist excess waits into
    standalone EventSemaphore instructions inserted just before the
    overloaded instruction on the same engine."""
    wid = [0]
    for f in nc.m.functions:
        for bbl in f.blocks:
            insts = bbl.instructions
            i = 0
            while i < len(insts):
                inst = insts[i]
                si = inst.sync_info
                if si is None:
                    i += 1
                    continue
                tname = type(inst).__name__
                cap = 2 if tname == "InstEventSemaphore" else 1
                waits = list(si.on_wait)
                if len(waits) <= cap:
                    i += 1
                    continue
                keep = waits[-cap:]
                hoist = waits[:-cap]
                inst.sync_info = _bass_rust.SyncInfo(
                    on_wait=keep, on_update=list(si.on_update)
                )
                pos = i
                for j in range(0, len(hoist), 2):
                    ev = mybir.InstEventSemaphore(
                        name=f"WX-{wid[0]}", ins=[], outs=[]
                    )
                    wid[0] += 1
                    ev.engine = inst.engine
                    ev.sync_info = _bass_rust.SyncInfo(
                        on_wait=hoist[j : j + 2], on_update=[]
                    )
                    insts.insert(pos, ev)
                    pos += 1
                    i += 1
                i += 1


class Cfg:
    def __init__(self, n_nodes=50000, n_edges=400000, ncore=8, d=64, h=8, nlayer=3):
        self.N = n_nodes
        self.E = n_edges
        self.NCORE = ncore
        self.D = d
        self.H = h
        self.L = nlayer
        self.PER = n_nodes // ncore          # owned real nodes per core
        self.BLK = 128
        self.NB = -(-self.PER // self.BLK)    # blocks per core
        self.PPAD = self.NB * self.BLK        # padded nodes per core
        self.HC = h * d                       # 512


# ---------------------------------------------------------------------------
# Host-side graph partitioning


def partition_edges(cfg, edge_index):
    """Returns (eidx [NCORE, NT, 3, 128] int32, TB) with per-(core,block)
    128-edge tiles.  Channel 0: padded-global src id, channel 1: padded-global
    dst id, channel 2: dst column within block as f32 bits (-1 for pad)."""
    N, PER, PPAD, BLK, NB = cfg.N, cfg.PER, cfg.PPAD, cfg.BLK, cfg.NB
    src = np.concatenate([edge_index[0], np.arange(N, dtype=np.int64)]).astype(np.int64)
    dst = np.concatenate([edge_index[1], np.arange(N, dtype=np.int64)]).astype(np.int64)
    srcp = (src // PER) * PPAD + src % PER
    owner = dst // PER
    loc = dst % PER
    blk = loc // BLK
    dcol = loc % BLK

    key = (owner * NB + blk).astype(np.int64)
    order = np.argsort(key, kind="stable")
    key_s = key[order]
    # per-(core,block) edge counts
    counts = np.bincount(key_s, minlength=cfg.NCORE * NB)
    # pad nodes (loc >= PER within the last block) get one fake self edge so
    # their softmax denominator is finite
    npad_node = PPAD - PER
    maxcnt = int(counts.max()) + (npad_node if npad_node else 0)
    TB = max(1, -(-maxcnt // 128))
    NT = NB * TB

    eidx = np.zeros((cfg.NCORE, NT, 3, 128), np.int32)
    eidx[:, :, 2, :] = np.float32(-1.0).view(np.int32)  # pad dcol = -1.0f
    starts = np.zeros(cfg.NCORE * NB + 1, np.int64)
    np.cumsum(counts, out=starts[1:])
    srcp_s = srcp[order]
    dstp_s = (owner * PPAD + loc)[order]
    dcol_s = dcol[order]
    for k in range(cfg.NCORE):
        for b in range(NB):
            s0, s1 = starts[k * NB + b], starts[k * NB + b + 1]
            cnt = int(s1 - s0)
            es = srcp_s[s0:s1]
            ed = dstp_s[s0:s1]
            ec = dcol_s[s0:s1].astype(np.float32)
            if b == NB - 1 and npad_node:
                # fake self edges for pad nodes
                pad_ids = k * PPAD + np.arange(PER, PPAD, dtype=np.int64)
                es = np.concatenate([es, pad_ids])
                ed = np.concatenate([ed, pad_ids])
                ec = np.concatenate(
                    [ec, np.arange(PER - (NB - 1) * BLK, BLK, dtype=np.float32)]
                )
                cnt += npad_node
            nslot = TB * 128
            fsrc = np.zeros(nslot, np.int32)
            fdst = np.zeros(nslot, np.int32)
            fcol = np.full(nslot, np.float32(-1.0).view(np.int32), np.int32)
            fsrc[:cnt] = es
            fdst[:cnt] = ed
            fcol[:cnt] = ec.view(np.int32)
            eidx[k, b * TB : (b + 1) * TB, 0, :] = fsrc.reshape(TB, 128)
            eidx[k, b * TB : (b + 1) * TB, 1, :] = fdst.reshape(TB, 128)
            eidx[k, b * TB : (b + 1) * TB, 2, :] = fcol.reshape(TB, 128)
    return eidx, TB


# ---------------------------------------------------------------------------
# Device program


def build_program(cfg, TB, trivial_affine=True, with_cc=True, ablate=(), reps=1):
    NB, BLK, D, H, HC, L = cfg.NB, cfg.BLK, cfg.D, cfg.H, cfg.HC, cfg.L
    NT = NB * TB
    PPAD = cfg.PPAD
    NPADG = cfg.NCORE * PPAD
    PER = cfg.PER
    NCORE = cfg.NCORE

    nc = bass.Bass("TRN2", target_bir_lowering=False, debug=False, num_devices=NCORE)

    xin = nc.dram_tensor("xin", [NPADG, D], BF16, kind="ExternalInput").ap()
    xown0 = nc.dram_tensor("xown0", [PPAD, D], BF16, kind="ExternalInput").ap()
    eidx = nc.dram_tensor("eidx", [NT, 3, 128], I32, kind="ExternalInput").ap()
    wc = nc.dram_tensor("wc", [L, 2 * D, HC], BF16, kind="ExternalInput").ap()
    att2 = nc.dram_tensor("att2", [L, 128, 2 * HC], BF16, kind="ExternalInput").ap()
    wrmn = nc.dram_tensor("wrmn", [L, D, D], BF16, kind="ExternalInput").ap()
    wcl = nc.dram_tensor("wcl", [L, 2 * D, H], BF16, kind="ExternalInput").ap()
    wrh = nc.dram_tensor("wrh", [L, D, HC], BF16, kind="ExternalInput").ap()
    wrlh = nc.dram_tensor("wrlh", [L, D, H], BF16, kind="ExternalInput").ap()
    id64 = nc.dram_tensor("id64", [D, D], BF16, kind="ExternalInput").ap()
    id128 = nc.dram_tensor("id128", [128, 128], BF16, kind="ExternalInput").ap()
    iotar = nc.dram_tensor("iotar", [128, 128], F32, kind="ExternalInput").ap()
    out = nc.dram_tensor("out", [PER, D], F32, kind="ExternalOutput").ap()

    with tile.TileContext(nc) as tc:
        import contextlib

        ctx = contextlib.ExitStack()
        with ctx:
            cpool = ctx.enter_context(tc.tile_pool(name="consts", bufs=1))
            dpool = ctx.enter_context(tc.tile_pool(name="dram", bufs=1, space="DRAM"))
            ppool = ctx.enter_context(tc.tile_pool(name="psum", bufs=1, space="PSUM"))
            spool = ctx.enter_context(tc.tile_pool(name="sbuf", bufs=10))
            gpool = ctx.enter_context(tc.tile_pool(name="grp", bufs=6))
            fpool = ctx.enter_context(tc.tile_pool(name="fin", bufs=3))

            # persistent constants
            id128_sb = cpool.tile([128, 128], BF16, tag="id128")
            nc.sync.dma_start(out=id128_sb[:], in_=id128[:])
            iota_sb = cpool.tile([128, 128], F32, tag="iotar")
            nc.sync.dma_start(out=iota_sb[:], in_=iotar[:])
            id64_sb = cpool.tile([D, D], BF16, tag="id64")
            nc.sync.dma_start(out=id64_sb[:], in_=id64[:])
            epsc = cpool.tile([128, 1], F32, tag="epsc")
            nc.gpsimd.memset(epsc[:], LN_EPS)
            wc_sb = []
            att2_sb = []
            wrmn_sb = []
            wcl_sb = []
            wrh_sb = []
            wrlh_sb = []
            for l in range(L):
                w = cpool.tile([2 * D, HC], BF16, tag=f"wc{l}")
                nc.sync.dma_start(out=w[:], in_=wc[l])
                wc_sb.append(w)
                a = cpool.tile([128, 2 * HC], BF16, tag=f"att2{l}")
                nc.sync.dma_start(out=a[:], in_=att2[l])
                att2_sb.append(a)
                m = cpool.tile([D, D], BF16, tag=f"wrmn{l}")
                nc.sync.dma_start(out=m[:], in_=wrmn[l])
                wrmn_sb.append(m)
                wl_ = cpool.tile([2 * D, H], BF16, tag=f"wcl{l}", name=f"wcl{l}")
                nc.sync.dma_start(out=wl_[:], in_=wcl[l])
                wcl_sb.append(wl_)
                wr_ = cpool.tile([D, HC], BF16, tag=f"wrh{l}", name=f"wrh{l}")
                nc.sync.dma_start(out=wr_[:], in_=wrh[l])
                wrh_sb.append(wr_)
                wrl_ = cpool.tile([D, H], BF16, tag=f"wrlh{l}", name=f"wrlh{l}")
                nc.sync.dma_start(out=wrl_[:], in_=wrlh[l])
                wrlh_sb.append(wrl_)

            # inter-layer DRAM buffers
            xsh = [
                dpool.tile([PPAD, D], BF16, tag=f"xsh{i}", name=f"xsh{i}")
                for i in range(L - 1)
            ]
            xg = [
                dpool.tile(
                    [NPADG, D], BF16, tag=f"xg{i}", name=f"xg{i}",
                    addr_space="Shared" if reps == 1 else "Local",
                )
                for i in range(L - 1)
            ]

            GSZ = GSZ_V

            def _block_start(l, b, xown_src):
                agg = ppool.tile([128, HC], F32, tag="agg", bufs=B_AGG)
                dx = ppool.tile([128, 2 * D], F32, tag="dx", bufs=B_DX)
                xo = fpool.tile([128, D], BF16, tag="xo")
                nc.sync.dma_start(
                    out=xo[:], in_=xown_src[b * BLK : (b + 1) * BLK]
                )
                tpx = ppool.tile([128, 128], BF16, tag="tps", bufs=B_TPS)
                nc.tensor.transpose(
                    out=tpx[:D, :], in_=xo[:], identity=id128_sb[:]
                )
                xoT = fpool.tile([D, 128], BF16, tag="xoT")
                nc.scalar.copy(out=xoT[:], in_=tpx[:D, :])
                return dict(agg=agg, dx=dx, xo=xo, xoT=xoT)

            def finalize(l, b, ctx_b):
                agg_ps = ctx_b["agg"]
                dx_ps = ctx_b["dx"]
                den_ps = dx_ps[:, :H]
                if "fin" in ablate:
                    y0 = fpool.tile([128, D], F32, tag="yf0")
                    nc.scalar.copy(out=y0[:], in_=agg_ps[:, :D])
                    y0b = fpool.tile([128, D], BF16, tag="yf0b")
                    nc.vector.tensor_copy(out=y0b[:, :H], in_=den_ps)
                    nc.gpsimd.memset(y0b[:, H:], 0)
                    if l < L - 1:
                        nc.sync.dma_start(out=xsh[l][b * BLK : (b + 1) * BLK], in_=y0b[:])
                    else:
                        rows = min(BLK, PER - b * BLK)
                        nc.sync.dma_start(out=out[b * BLK : b * BLK + rows], in_=y0[:rows, :])
                    return
                recip = fpool.tile([128, H], F32, tag="recip")
                nc.vector.reciprocal(out=recip[:], in_=den_ps)
                recip8 = fpool.tile([128, H], F32, tag="recip8")
                nc.vector.tensor_scalar_mul(
                    out=recip8[:], in0=recip[:], scalar1=1.0 / H
                )
                tmp = fpool.tile([128, HC], F32, tag="ftmp")
                rb = recip8[:].unsqueeze(1).to_broadcast([128, D, H])
                nc.vector.tensor_tensor(
                    out=tmp[:], in0=agg_ps[:].rearrange("p (c h) -> p c h", h=H),
                    in1=rb, op=ALU.mult
                )
                v = fpool.tile([128, D], F32, tag="v")
                tmp_t = tmp[:].rearrange("p (c h) -> p c h", h=H)
                nc.vector.tensor_reduce(
                    out=v[:], in_=tmp_t, axis=mybir.AxisListType.X, op=ALU.add
                )
                xoT = ctx_b["xoT"]
                xadj = dx_ps[:, D : 2 * D]
                nc.tensor.matmul(
                    out=xadj, lhsT=xoT[:], rhs=wrmn_sb[l][:],
                    start=True, stop=False, skip_group_check=True,
                )
                nc.tensor.matmul(
                    out=xadj, lhsT=xoT[:], rhs=id64_sb[:],
                    start=False, stop=True, skip_group_check=True,
                )
                t3 = fpool.tile([128, D], F32, tag="t3")
                nc.vector.tensor_tensor(
                    out=t3[:], in0=v[:], in1=xadj[:], op=ALU.add
                )
                scr = fpool.tile([128, D], F32, tag="scr")
                mu = fpool.tile([128, 1], F32, tag="mu")
                nc.scalar.activation(
                    out=scr[:], in_=t3[:], func=AF.Copy, scale=1.0 / D, accum_out=mu[:]
                )
                cen = fpool.tile([128, D], F32, tag="cen")
                nc.vector.tensor_scalar(
                    out=cen[:], in0=t3[:], scalar1=mu[:, 0:1], scalar2=None,
                    op0=ALU.subtract,
                )
                scr2 = fpool.tile([128, D], F32, tag="scr2")
                vs = fpool.tile([128, 1], F32, tag="vs")
                nc.scalar.activation(
                    out=scr2[:], in_=cen[:], func=AF.Square, accum_out=vs[:]
                )
                lnv = fpool.tile([128, 1], F32, tag="lnv")
                nc.scalar.activation(
                    out=lnv[:], in_=vs[:], func=AF.Ln, scale=1.0 / D,
                    bias=epsc[:, 0:1],
                )
                rs = fpool.tile([128, 1], F32, tag="rs")
                nc.scalar.activation(out=rs[:], in_=lnv[:], func=AF.Exp,
                                     scale=-0.5)
                xn = fpool.tile([128, D], F32, tag="xn")
                nc.vector.tensor_scalar(
                    out=xn[:], in0=cen[:], scalar1=rs[:, 0:1], scalar2=None,
                    op0=ALU.mult,
                )
                if l < L - 1:
                    y = fpool.tile([128, D], BF16, tag="yb")
                    nc.scalar.activation(out=y[:], in_=xn[:], func=AF.Relu)
                    nc.sync.dma_start(
                        out=xsh[l][b * BLK : (b + 1) * BLK], in_=y[:]
                    )
                else:
                    y = fpool.tile([128, D], F32, tag="yf")
                    nc.scalar.activation(out=y[:], in_=xn[:], func=AF.Relu)
                    rows = min(BLK, PER - b * BLK)
                    if rows > 0:
                        nc.sync.dma_start(
                            out=out[b * BLK : b * BLK + rows], in_=y[:rows, :]
                        )

            for _rep in range(reps):
              for l in range(L):
                xsrc = xin if l == 0 else xg[l - 1][:]
                xown_src = xown0 if l == 0 else xsh[l - 1][:]
                blkctx = {}
                g = 0
                while g < NT:
                    tl = list(range(g, min(g + GSZ, NT)))
                    n = len(tl)
                    ei = spool.tile([128, 3 * n], I32, tag="ei")
                    nc.sync.dma_start(
                        out=ei[:],
                        in_=eidx[tl[0] : tl[0] + n].rearrange("t c p -> p (t c)"),
                    )
                    s2 = ppool.tile([128, HC * n], F32, tag="s2", bufs=B_S2)
                    lin = ppool.tile([128, H * n], F32, tag="lin", bufs=B_LIN)
                    indT = []
                    for j, t in enumerate(tl):
                        b, tt = divmod(t, TB)
                        if tt == 0:
                            blkctx[b] = _block_start(l, b, xown_src)
                        xsd = spool.tile([128, 128], BF16, tag="xsd")
                        if "gather" in ablate:
                            nc.gpsimd.memset(xsd[:, 0:1], 0)
                        else:
                            nc.gpsimd.indirect_dma_start(
                                out=xsd[:, 0:D], out_offset=None, in_=xsrc,
                                in_offset=bass.IndirectOffsetOnAxis(
                                    ap=ei[:, 3 * j : 3 * j + 1], axis=0
                                ),
                            )
                            nc.gpsimd.indirect_dma_start(
                                out=xsd[:, D : 2 * D], out_offset=None, in_=xsrc,
                                in_offset=bass.IndirectOffsetOnAxis(
                                    ap=ei[:, 3 * j + 1 : 3 * j + 2], axis=0
                                ),
                            )
                        it = spool.tile([128, 128], BF16, tag="indT")
                        nc.vector.tensor_scalar(
                            out=it[:], in0=iota_sb[:],
                            scalar1=ei[:, 3 * j + 2 : 3 * j + 3].bitcast(F32),
                            scalar2=None, op0=ALU.is_equal,
                        )
                        indT.append(it)
                        tp = ppool.tile([128, 128], BF16, tag="tps", bufs=B_TPS)
                        nc.tensor.transpose(
                            out=tp[:], in_=xsd[:], identity=id128_sb[:]
                        )
                        lhsT = spool.tile([128, 128], BF16, tag="lhsT")
                        nc.scalar.copy(out=lhsT[:], in_=tp[:])
                        nc.tensor.matmul(
                            out=s2[:, HC * j : HC * (j + 1)], lhsT=lhsT[:],
                            rhs=wc_sb[l][:], start=True, stop=True,
                            skip_group_check=True,
                        )
                        nc.tensor.matmul(
                            out=lin[:, H * j : H * (j + 1)], lhsT=lhsT[:],
                            rhs=wcl_sb[l][:], start=True, stop=True,
                            skip_group_check=True,
                        )
                    # one bf16 copy of s (c-major head layout) feeds both
                    # the score and aggregation paths; relu runs on DVE in
                    # the 2x mode
                    s2sb = gpool.tile([128, HC * n], BF16, tag="s2sb")
                    nc.scalar.copy(out=s2sb[:], in_=s2[:, : HC * n])
                    m2 = gpool.tile([128, HC * n], BF16, tag="m2")
                    nc.vector.tensor_scalar(
                        out=m2[:], in0=s2sb[:], scalar1=0.0, scalar2=None,
                        op0=ALU.max,
                    )
                    mw2 = gpool.tile([128, HC * n], BF16, tag="mw2")
                    nc.vector.tensor_tensor(
                        out=mw2[:], in0=m2[:], in1=att2_sb[l][:, : HC * n],
                        op=ALU.mult,
                    )
                    e2r = gpool.tile([128, H * n], F32, tag="e2r")
                    if True:
                        nc.vector.tensor_reduce(
                            out=e2r[:],
                            in_=mw2[:].rearrange("p (t c h) -> p t h c", t=n, h=H),
                            axis=mybir.AxisListType.X, op=ALU.add,
                        )
                    e2 = gpool.tile([128, H * n], F32, tag="e2")
                    nc.vector.tensor_tensor(
                        out=e2[:], in0=e2r[:], in1=lin[:, : H * n], op=ALU.add
                    )
                    ex2 = gpool.tile([128, H * n], BF16, tag="ex2")
                    nc.scalar.activation(out=ex2[:], in_=e2[:], func=AF.Exp)
                    w2 = gpool.tile([128, HC * n], BF16, tag="w2")
                    exb = (
                        ex2[:]
                        .rearrange("p (t h) -> p t h", t=n)
                        .unsqueeze(2)
                        .to_broadcast([128, n, D, H])
                    )
                    nc.vector.tensor_tensor(
                        out=w2[:], in0=s2sb[:], in1=exb, op=ALU.mult
                    )
                    for j, t in enumerate(tl):
                        b, tt = divmod(t, TB)
                        ctx_b = blkctx[b]
                        nc.tensor.matmul(
                            out=ctx_b["agg"][:], lhsT=indT[j][:],
                            rhs=w2[:, HC * j : HC * (j + 1)],
                            start=(tt == 0), stop=(tt == TB - 1),
                            skip_group_check=True,
                        )
                        nc.tensor.matmul(
                            out=ctx_b["dx"][:, :H], lhsT=indT[j][:],
                            rhs=ex2[:, H * j : H * (j + 1)],
                            start=(tt == 0), stop=(tt == TB - 1),
                            skip_group_check=True,
                        )
                        if tt == TB - 1:
                            finalize(l, b, ctx_b)
                            del blkctx[b]
                    g += n
                if l < L - 1 and with_cc:
                    nc.gpsimd.collective_compute(
                        "AllGather", ALU.bypass,
                        replica_groups=[list(range(NCORE))],
                        ins=[xsh[l].opt()], outs=[xg[l].opt()],
                    )
    return nc


# ---------------------------------------------------------------------------
# Host wrapper


def prep_inputs(cfg, x, Wl, Wr, att, edge_index):
    bf = ml_dtypes.bfloat16
    D, H, HC, L = cfg.D, cfg.H, cfg.HC, cfg.L
    PER, PPAD = cfg.PER, cfg.PPAD
    eidx, TB = partition_edges(cfg, edge_index.astype(np.int64))

    xpad = np.zeros((cfg.NCORE * PPAD, D), np.float32)
    for k in range(cfg.NCORE):
        xpad[k * PPAD : k * PPAD + PER] = x[k * PER : (k + 1) * PER]
    xpad_bf = xpad.astype(bf)

    # c-major head layout: output column (c*H + h) holds head h, channel c
    wc = np.concatenate([Wl, Wr], axis=1).reshape(L, 2 * D, H, D)
    wc = np.ascontiguousarray(wc.transpose(0, 1, 3, 2)).reshape(L, 2 * D, HC)
    wc = wc.astype(bf)
    attf = (0.8 * att.transpose(0, 2, 1)).reshape(L, 1, HC)
    att2 = np.broadcast_to(
        np.concatenate([attf, attf], axis=2), (L, 128, 2 * HC)
    ).astype(bf)
    # lin[d, h] = 0.2 * sum_c Wc[d, h*CH+c] * att[h, c]
    wcf = np.concatenate([Wl, Wr], axis=1).reshape(L, 2 * D, H, D)
    wcl = (0.2 * np.einsum('ldhc,lhc->ldh', wcf, att)).astype(bf)  # [L, 128, 8]
    wrmn = (-Wr.reshape(L, D, H, D).mean(axis=2)).astype(bf)  # negated head-mean
    id64 = np.eye(D, dtype=np.float32).astype(bf)
    id128 = np.eye(128, dtype=np.float32).astype(bf)
    iotar = np.broadcast_to(
        np.arange(128, dtype=np.float32)[None, :], (128, 128)
    ).copy()

    wrh = Wr.astype(bf)                                   # [L, 64, 512]
    wrlh = np.ascontiguousarray(wcl[:, D:, :])            # [L, 64, 8]
    shared = dict(
        wc=np.ascontiguousarray(wc), att2=np.ascontiguousarray(att2),
        wcl=np.ascontiguousarray(wcl), wrh=np.ascontiguousarray(wrh),
        wrlh=wrlh,
        wrmn=np.ascontiguousarray(wrmn), id64=id64, id128=id128, iotar=iotar,
        xin=xpad_bf,
    )
    in_maps = []
    for k in range(cfg.NCORE):
        m = dict(shared)
        m["eidx"] = np.ascontiguousarray(eidx[k])
        m["xown0"] = np.ascontiguousarray(xpad_bf[k * PPAD : (k + 1) * PPAD])
        in_maps.append(m)
    return in_maps, TB


_CACHE = {}


def _get_program(cfg_key, cfg, TB):
    key = (cfg_key, TB)
    if key not in _CACHE:
        nc = build_program(cfg, TB)
        split_excess_waits(nc)  # walrus-only fixup; breaks CoreSim bookkeeping
        _CACHE[key] = nc
    return _CACHE[key]


def kernel(x, Wl, Wr, att, b, gamma, beta, edge_index, _return_extras=False):
    x = np.asarray(x, np.float32)
    Wl = np.asarray(Wl, np.float32)
    Wr = np.asarray(Wr, np.float32)
    att = np.asarray(att, np.float32)
    edge_index = np.asarray(edge_index)
    assert np.all(np.asarray(b) == 0.0), "nonzero bias not supported"
    assert np.all(np.asarray(gamma) == 1.0), "non-unit gamma not supported"
    assert np.all(np.asarray(beta) == 0.0), "nonzero beta not supported"

    cfg = Cfg(
        n_nodes=x.shape[0], n_edges=edge_index.shape[1], d=x.shape[1],
        h=att.shape[1], nlayer=Wl.shape[0],
    )
    in_maps, TB = prep_inputs(cfg, x, Wl, Wr, att, edge_index)
    nc = _get_program((cfg.N, cfg.E, cfg.D, cfg.H, cfg.L), cfg, TB)
    res = run_bass_kernel_spmd(nc, in_maps, core_ids=list(range(cfg.NCORE)))
    outp = np.concatenate([res.results[k]["out"] for k in range(cfg.NCORE)], axis=0)
    outp = outp.astype(np.float32)
    if _return_extras:
        return outp, res
    return outp

